# revision 67
# baseline (speedup 1.0000x reference)
"""Trainium2 Bass kernel for single-head self-attention (B=2, S=4096, D=1024).

reference:
    q = x @ Wq; k = x @ Wk; v = x @ Wv          # [B,S,D]
    energy = einsum('bid,bjd->bij', q, k) * 8.0  # SCALE = sqrt(64)
    attn = softmax(energy, axis=-1)
    out = einsum('bij,bjd->bid', attn, v) @ Wo

Weight folding (associativity): energy = x @ (Wq Wk^T) @ x^T and
out = attn @ (x @ (Wv Wo)), so the host precomputes M = Wq @ Wk^T and
W' = Wv @ Wo once (fp64) and the device only runs TWO projections
(G = x@M, V' = x@W') plus the two S^2-sized attention matmuls -- the
separate K projection and the output projection disappear.

Two SPMD launches over 8 cores (= 2 batches x 4 query-blocks of 1024):
  phase 1: each core computes G / V' for its own 1024 rows; the host
           gathers V' shards (and pre-casts x^T to fp16) per batch.
  phase 2: each core computes softmax(G_blk @ x^T * 8) @ V' for its
           1024 queries against the full batch; output rows come out
           of the P@V' accumulation directly.

Precision: logits have std ~256 (SCALE multiplies), so the logit path
needs much better than bf16 input precision.  Everything runs in fp16
(2^-11 input rounding): fp16 products are exact in fp32 PSUM
accumulation, so each matmul only contributes its operands' rounding.
Four independent 2^-11 roundings on the logit path (x and M into G,
the G store, the x^T copy) give ~0.14 std of logit noise -> ~8e-3
output rel err vs the 2e-2 gate.  V'/P also run fp16.

Phase-2 layout: x^T (fp16, 64KB/part) and V' (fp16, 64KB/part) are
SBUF-resident, so after the initial load the attention sweep runs with
no input DMA.  A dma_start occupies its issuing queue through the
whole transfer, so queue assignment is part of the schedule: bulk
loads ride SP in consumption order (racing queues invert bus
priority), the ACT queue handles E psum->SBUF copies + the softmax exp
chain in dependency order (GPSIMD cannot touch PSUM on TRN2), and DVE
takes the reduces, P^T copies, and 1/l scales.
"""

import numpy as np
import ml_dtypes

B, S, D = 2, 4096, 1024
BLK = 1024          # queries per core
SCALE = 8.0         # HEAD_DIM ** 0.5 = sqrt(64)
NK = D // 128       # 8 k-tiles over the feature dim
NT = S // 128       # 32 j-tiles over keys
NI = BLK // 128     # 8 i-tiles over this core's queries
NJB = S // 512      # 8 key blocks of 512
BF16 = ml_dtypes.bfloat16

_cache = {}


def _build_phase1():
    """G = x@M and V' = x@W' for this core's 1024 rows (fp16 single-pass)."""
    import concourse.mybir as mybir
    from concourse import bacc
    from concourse.tile import TileContext

    FP16 = mybir.dt.float16
    FP32 = mybir.dt.float32
    FP32R = mybir.dt.float32r
    DBF = mybir.dt.bfloat16

    nc = bacc.Bacc("TRN2", target_bir_lowering=False, debug=False, num_devices=8)

    xt = nc.dram_tensor("xt", [D, BLK], FP16, kind="ExternalInput")  # rows.T
    wm = nc.dram_tensor("wm", [D, D], FP16, kind="ExternalInput")    # Wq@Wk^T
    wvo = nc.dram_tensor("wvo", [D, D], FP16, kind="ExternalInput")  # Wv@Wo
    gt = nc.dram_tensor("gt", [D, BLK], FP16, kind="ExternalOutput")
    vo = nc.dram_tensor("vo", [NI, 128, D], FP16, kind="ExternalOutput")

    with TileContext(nc) as tc:
      with (
          tc.tile_pool(name="xp", bufs=1) as xp,
          tc.tile_pool(name="wp", bufs=1) as wp,
          tc.tile_pool(name="gps", bufs=4, space="PSUM") as gps,
          tc.tile_pool(name="gst", bufs=1) as gstp,
          tc.tile_pool(name="vps", bufs=2, space="PSUM") as vps,
          tc.tile_pool(name="wup", bufs=2, space="PSUM") as wup,
          tc.tile_pool(name="vsb", bufs=3) as vsbp,
          tc.tile_pool(name="ztp", bufs=1) as ztp,
      ):
        # warm the PE clock during the DMA lead-in (HAM reaches 2.4GHz
        # after ~3us of sustained activity)
        zt = ztp.tile([128, 128], FP16, name="zt", tag="zt")
        nc.gpsimd.memset(zt, 0.0)
        for w in range(30):
            wps = wup.tile([128, 128], FP32, name=f"wu{w}", tag="wu")
            nc.tensor.matmul(wps, lhsT=zt, rhs=zt, start=True, stop=True)
        # all loads on the SP queue in consumption order: the shared DMA
        # bus serializes transfers, so a single queue in priority order
        # beats spreading (racing queues invert priorities)
        xt_r = xt[:, :].rearrange("(n p) s -> p n s", p=128)
        x_sb = xp.tile([128, NK, BLK], FP16, name="x_sb", tag="x_sb")
        wm_sb = wp.tile([128, NK, D], FP16, name="wm_sb", tag="wm_sb")
        wm_r = wm[:, :].rearrange("(n p) d -> p n d", p=128)
        nc.sync.dma_start(x_sb[:, :, 0:256], xt_r[:, :, 0:256])
        nc.sync.dma_start(wm_sb[:, :, 0:256], wm_r[:, :, 0:256])
        nc.sync.dma_start(x_sb[:, :, 256:512], xt_r[:, :, 256:512])
        nc.sync.dma_start(wm_sb[:, :, 256:512], wm_r[:, :, 256:512])
        nc.sync.dma_start(x_sb[:, :, 512:BLK], xt_r[:, :, 512:BLK])
        nc.sync.dma_start(wm_sb[:, :, 512:768], wm_r[:, :, 512:768])
        nc.sync.dma_start(wm_sb[:, :, 768:D], wm_r[:, :, 768:D])
        wv_sb = wp.tile([128, NK, D], FP16, name="wv_sb", tag="wv_sb")
        wvo_r = wvo[:, :].rearrange("(n p) d -> p n d", p=128)
        nc.sync.dma_start(wv_sb[:, :, 0:512], wvo_r[:, :, 0:512])
        nc.sync.dma_start(wv_sb[:, :, 512:D], wvo_r[:, :, 512:D])

        # G blocks emitted in DMA-supply order: each group becomes runnable
        # as one more of the loads above lands.
        K_ORDER = [
            (0, 0), (0, 1),
            (1, 0), (1, 1),
            (0, 2), (0, 3), (1, 2), (1, 3),
            (2, 0), (2, 1), (2, 2), (2, 3),
            (0, 4), (0, 5), (1, 4), (1, 5), (2, 4), (2, 5),
            (0, 6), (0, 7), (1, 6), (1, 7), (2, 6), (2, 7),
        ]
        NBS = ((0, 256), (256, 256), (512, 512))
        st = []
        for m in range(NK):
            st.append(gstp.tile([128, BLK], FP16, name=f"gs{m}", tag=f"s{m}"))
        for (nb, m) in K_ORDER:
            n0, nw = NBS[nb]
            nsl = slice(n0, n0 + nw)
            msl = slice(m * 128, (m + 1) * 128)
            ps = gps.tile([128, 512], FP32, name=f"gp{n0}_{m}", tag="ps")
            for k in range(NK):
                nc.tensor.matmul(ps[:, 0:nw], lhsT=wm_sb[:, k, msl],
                                 rhs=x_sb[:, k, nsl],
                                 start=(k == 0), stop=(k == NK - 1))
            nc.vector.tensor_copy(st[m][:, nsl], ps[:, 0:nw])
        for m in range(NK):
            nc.sync.dma_start(gt[m * 128:(m + 1) * 128, :], st[m])

        for j in range(NI):
            jsl = slice(j * 128, (j + 1) * 128)
            vt = vsbp.tile([128, D], FP16, name=f"vt{j}", tag="vt")
            for db in range(2):
                ps = vps.tile([128, 512], FP32, name=f"vps{j}_{db}", tag="vps")
                for k in range(NK):
                    nc.tensor.matmul(
                        ps, lhsT=x_sb[:, k, jsl],
                        rhs=wv_sb[:, k, db * 512:(db + 1) * 512],
                        start=(k == 0), stop=(k == NK - 1),
                    )
                nc.vector.tensor_copy(vt[:, db * 512:(db + 1) * 512], ps)
                nc.scalar.dma_start(vo[j][:, db * 512:(db + 1) * 512],
                                    vt[:, db * 512:(db + 1) * 512])
    nc.compile()
    return nc


def _build_phase2():
    """softmax(G_blk @ x^T * 8) @ V' for this core's 1024 queries."""
    import concourse.mybir as mybir
    from concourse import bacc
    from concourse.tile import TileContext
    from concourse.masks import make_identity

    FP16 = mybir.dt.float16
    FP32 = mybir.dt.float32
    DBF = mybir.dt.bfloat16
    Exp = mybir.ActivationFunctionType.Exp
    Copy = mybir.ActivationFunctionType.Copy
    AX = mybir.AxisListType.X

    nc = bacc.Bacc("TRN2", target_bir_lowering=False, debug=False, num_devices=8)

    xth = nc.dram_tensor("xth", [D, S], FP16, kind="ExternalInput")
    # per-i-tile partition-major G: [i, p, n, f] = gt[n*128+p, i*128+f]
    gt2 = nc.dram_tensor("gt2", [NI, 128, NK, 128], FP16, kind="ExternalInput")
    # partition-major V': [p, t, d] = V'[t*128+p, d]
    vin = nc.dram_tensor("vin", [128, NT, D], FP16, kind="ExternalInput")
    y = nc.dram_tensor("y", [BLK, D], FP16, kind="ExternalOutput")

    from contextlib import ExitStack
    with TileContext(nc) as tc:
        with ExitStack() as stack:
            constp = stack.enter_context(tc.tile_pool(name="const", bufs=1))
            ident = constp.tile([128, 128], FP16)
            make_identity(nc, ident)

            ktp = stack.enter_context(tc.tile_pool(name="ktp", bufs=1))
            qtp = stack.enter_context(tc.tile_pool(name="qtp", bufs=4))
            vvp = stack.enter_context(tc.tile_pool(name="vvp", bufs=1))
            epsp = stack.enter_context(tc.tile_pool(name="eps", bufs=2, space="PSUM"))
            tpsp = stack.enter_context(tc.tile_pool(name="tps", bufs=2, space="PSUM"))
            opsp = stack.enter_context(tc.tile_pool(name="ops", bufs=2, space="PSUM"))
            smp = stack.enter_context(tc.tile_pool(name="smp", bufs=2))
            esp = stack.enter_context(tc.tile_pool(name="esp", bufs=5))
            pp = stack.enter_context(tc.tile_pool(name="pp", bufs=3))
            ptp = stack.enter_context(tc.tile_pool(name="ptp", bufs=1))
            obp = stack.enter_context(tc.tile_pool(name="obp", bufs=2))

            gv_t = [None] * NI

            def gv(i):
                return gv_t[i]

            gv_t[0] = qtp.tile([128, NK, 128], FP16, name="gv0", tag="gv")
            nc.sync.dma_start(gv_t[0], gt2[0])
            # x^T as one [128, NK, S] tile: each column chunk is a single
            # batched DMA covering all 8 k-rows (full bus bandwidth, one
            # queue slot); everything rides SP in consumption order
            xth_r = xth[:, :].rearrange("(n p) s -> p n s", p=128)
            xth_all = ktp.tile([128, NK, S], FP16, name="xth_all", tag="xth")
            xth_sb = [xth_all[:, m, :] for m in range(NK)]
            vv_all = vvp.tile([128, NT, D], FP16, name="vv_all", tag="vv")
            nc.sync.dma_start(xth_all[:, :, 0:512], xth_r[:, :, 0:512])
            for i in range(1, 4):
                gv_t[i] = qtp.tile([128, NK, 128], FP16, name=f"gv{i}",
                                   tag="gv")
                nc.sync.dma_start(gv_t[i], gt2[i])
            nc.sync.dma_start(xth_all[:, :, 512:1024], xth_r[:, :, 512:1024])
            nc.sync.dma_start(xth_all[:, :, 1024:1536], xth_r[:, :, 1024:1536])
            nc.sync.dma_start(xth_all[:, :, 1536:2560], xth_r[:, :, 1536:2560])
            nc.sync.dma_start(xth_all[:, :, 2560:3584], xth_r[:, :, 2560:3584])
            nc.sync.dma_start(xth_all[:, :, 3584:4096], xth_r[:, :, 3584:4096])
            nc.sync.dma_start(vv_all[:, 0:16, :], vin[:, 0:16, :])
            nc.sync.dma_start(vv_all[:, 16:NT, :], vin[:, 16:NT, :])

            st_mx8 = [None, None]
            eq_t = [[None, None], [None, None]]   # [i%2][half]

            def e_block(i, jb):
                sl = slice(jb * 512, (jb + 1) * 512)
                ps = epsp.tile([128, 512], FP32, name=f"eps{i}_{jb}", tag="eps")
                for k in range(NK):
                    nc.tensor.matmul(ps, lhsT=gv(i)[:, k, :],
                                     rhs=xth_sb[k][:, sl],
                                     start=(k == 0), stop=(k == NK - 1))
                half = jb % 4
                nc.scalar.activation(
                    eq_t[i % 2][jb // 4][:, half * 512:(half + 1) * 512], ps,
                    Copy)
                nc.vector.reduce_max(st_mx8[i % 2][:, jb:jb + 1], ps, axis=AX)

            def softmax_issue(i):
                """Global max + exp chain (DVE stats + ACT exps) for i."""
                mx8 = st_mx8[i % 2]
                mrow = smp.tile([128, 1], FP32, name=f"mrow{i}", tag="mrow")
                nc.vector.reduce_max(mrow, mx8, axis=AX)
                negm = smp.tile([128, 1], FP32, name=f"negm{i}", tag="negm")
                nc.vector.tensor_scalar_mul(negm, mrow, -SCALE)
                # P in two half-row tiles so the next i's exp can start as
                # soon as the first half's transposes have consumed it
                p_h = [pp.tile([128, S // 2], FP16, name=f"p{i}_{h}", tag="p")
                       for h in range(2)]
                lp8 = smp.tile([128, NJB], FP32, name=f"lp8_{i}", tag="lp8")
                for jb in range(NJB):
                    half = jb % 4
                    nc.scalar.activation(
                        p_h[jb // 4][:, (jb % 4) * 512:(jb % 4) * 512 + 512],
                        eq_t[i % 2][jb // 4][:, half * 512:(half + 1) * 512],
                        Exp, bias=negm, scale=SCALE,
                        accum_out=lp8[:, jb:jb + 1],
                    )
                lrow = smp.tile([128, 1], FP32, name=f"lrow{i}", tag="lrow")
                nc.vector.reduce_sum(lrow, lp8, axis=AX)
                linv = smp.tile([128, 1], FP32, name=f"linv{i}", tag="linv")
                nc.vector.reciprocal(linv, lrow)
                return p_h, linv

            _uid = [0]
            pt_sb = ptp.tile([128, NT, 128], FP16, name="pt", tag="pt")

            def pt_group(p_h, g):
                """Transpose 4 P tiles (4g..4g+3) via one psum bank group."""
                _uid[0] += 1
                tp = tpsp.tile([128, 512], FP16, name=f"tpg{_uid[0]}", tag="tp")
                for w in range(4):
                    tl = (4 * g + w) % 16
                    nc.tensor.transpose(tp[:, w * 128:(w + 1) * 128],
                                        p_h[g // 4][:, tl * 128:(tl + 1) * 128],
                                        ident)
                nc.vector.tensor_copy(
                    pt_sb[:, 4 * g:4 * g + 4, :].rearrange("p t f -> p (t f)"), tp)

            def pv_sweep(i, p_h, linv, last=False, next_ph=None,
                         skip_pre=False):
                """P^T + P@V' + 1/l scale + row store for i-tile i.

                P^T groups are issued two ahead of their consuming matmuls
                so the psum->pt copy latency hides under PE work.  For the
                last i-tile the two output halves run as separate t-sweeps
                so half 0 stores while half 1 computes (shorter drain).
                """
                op0 = opsp.tile([128, 512], FP32, name=f"op0_{i}", tag="op0")
                op1 = opsp.tile([128, 512], FP32, name=f"op1_{i}", tag="op1")
                osb = obp.tile([128, D], FP16, name=f"osb{i}", tag="osb")
                halves = ((op0, 0, 512), (op1, 512, D)) if last else None
                if last:
                    if not skip_pre:
                        pt_group(p_h, 0)
                        pt_group(p_h, 1)
                    for (op, d0, d1) in halves:
                        for t in range(NT):
                            g = t // 4
                            if t % 4 == 0 and g + 2 < NT // 4 and op is op0:
                                pt_group(p_h, g + 2)
                            nc.tensor.matmul(op, lhsT=pt_sb[:, t, :],
                                             rhs=vv_all[:, t, d0:d1],
                                             start=(t == 0), stop=(t == NT - 1))
                        nc.scalar.activation(osb[:, d0:d1], op, Copy,
                                             scale=linv)
                        nc.sync.dma_start(y[i * 128:(i + 1) * 128, d0:d1],
                                          osb[:, d0:d1])
                    return
                if not skip_pre:
                    pt_group(p_h, 0)
                    pt_group(p_h, 1)
                for g in range(NT // 4):
                    if g + 2 < NT // 4:
                        pt_group(p_h, g + 2)
                    for w in range(4):
                        t = 4 * g + w
                        nc.tensor.matmul(op0, lhsT=pt_sb[:, t, :],
                                         rhs=vv_all[:, t, 0:512],
                                         start=(t == 0), stop=(t == NT - 1))
                        nc.tensor.matmul(op1, lhsT=pt_sb[:, t, :],
                                         rhs=vv_all[:, t, 512:D],
                                         start=(t == 0), stop=(t == NT - 1))
                # hoist the NEXT sweep's first two P^T groups here so its
                # first matmuls don't wait on the transpose-copy pipeline
                if next_ph is not None:
                    pt_group(next_ph, 0)
                    pt_group(next_ph, 1)
                nc.scalar.activation(osb[:, 0:512], op0, Copy, scale=linv)
                nc.scalar.activation(osb[:, 512:D], op1, Copy, scale=linv)
                nc.sync.dma_start(y[i * 128:(i + 1) * 128, :], osb)

            sm = [None] * NI

            def E_tile(i, jbs):
                if jbs[0] == 0:
                    st_mx8[i % 2] = smp.tile([128, NJB], FP32, name=f"mx8_{i}",
                                             tag=f"mx8{i % 2}")
                for jb in jbs:
                    if jb % 4 == 0:
                        eq_t[i % 2][jb // 4] = esp.tile(
                            [128, 2048], FP32, name=f"e{i}_{jb // 4}", tag="e")
                    e_block(i, jb)

            # head: E(0)/E(1) interleaved jb-major over the x^T chunk
            # supply; E(0) finishes first so exp(0) starts early
            for jb in range(6):
                E_tile(0, [jb])
                E_tile(1, [jb])
            E_tile(0, [6, 7])
            sm[0] = softmax_issue(0)
            E_tile(1, [6, 7])
            # lag-2 pipeline: E(i); exps(i-1) (after E(i)'s psum copies so
            # the ACT queue drains in dependency order); sweep(i-2)
            for i in range(2, NI):
                E_tile(i, list(range(NJB)))
                if i + 2 < NI:
                    gv_t[i + 2] = qtp.tile([128, NK, 128], FP16,
                                           name=f"gv{i + 2}", tag="gv")
                    nc.sync.dma_start(gv_t[i + 2], gt2[i + 2])
                sm[i - 1] = softmax_issue(i - 1)
                pv_sweep(i - 2, sm[i - 2][0], sm[i - 2][1],
                         next_ph=sm[i - 1][0], skip_pre=(i > 2))
            sm[NI - 1] = softmax_issue(NI - 1)
            pv_sweep(NI - 2, sm[NI - 2][0], sm[NI - 2][1],
                     next_ph=sm[NI - 1][0], skip_pre=True)
            pv_sweep(NI - 1, sm[NI - 1][0], sm[NI - 1][1], last=True,
                     skip_pre=True)
    nc.compile()
    return nc


def _get_programs():
    if "nc1" not in _cache:
        _cache["nc1"] = _build_phase1()
        _cache["nc2"] = _build_phase2()
    return _cache["nc1"], _cache["nc2"]


def kernel(x, Wq, Wk, Wv, Wo):
    from concourse.bass_utils import run_bass_kernel_spmd

    nc1, nc2 = _get_programs()

    x = np.asarray(x, dtype=np.float32)
    # fold the weights once on the host (associativity):
    #   energy = x (Wq Wk^T) x^T ;  out = attn (x (Wv Wo))
    wm = (np.asarray(Wq, np.float64) @ np.asarray(Wk, np.float64).T
          ).astype(np.float16)
    wvo = (np.asarray(Wv, np.float64) @ np.asarray(Wo, np.float64)
           ).astype(np.float16)

    # ---- phase 1: per-core row slices ----
    in1 = []
    for c in range(8):
        b, i = divmod(c, 4)
        rows = x[b, i * BLK:(i + 1) * BLK, :]           # [BLK, D]
        in1.append({
            "xt": np.ascontiguousarray(rows.T.astype(np.float16)),
            "wm": wm, "wvo": wvo,
        })
    res1 = run_bass_kernel_spmd(nc1, in1, list(range(8))).results

    # ---- host gather of V' shards; pre-cast x^T per batch ----
    xth_full, v_full = [], []
    for b in range(B):
        xth_full.append(np.ascontiguousarray(
            x[b].T.astype(np.float16)))                  # [D, S]
        v = np.concatenate(
            [res1[b * 4 + i]["vo"] for i in range(4)], axis=0)    # [NT, 128, D]
        v_full.append(np.ascontiguousarray(v.transpose(1, 0, 2)))  # [128, NT, D]

    # ---- phase 2 ----
    in2 = []
    for c in range(8):
        b, i = divmod(c, 4)
        gstack = res1[c]["gt"].reshape(NK, 128, NI, 128)  # [n, p, i, f]
        in2.append({
            "xth": xth_full[b], "vin": v_full[b],
            "gt2": np.ascontiguousarray(gstack.transpose(2, 1, 0, 3)),
        })
    res2 = run_bass_kernel_spmd(nc2, in2, list(range(8))).results

    out = np.empty((B, S, D), dtype=np.float32)
    for c in range(8):
        b, i = divmod(c, 4)
        out[b, i * BLK:(i + 1) * BLK, :] = res2[c]["y"].astype(np.float32)
    return out
